# revision 1
# baseline (speedup 1.0000x reference)
"""Trainium2 Bass kernel for nn_BitfieldLinear (vq_codebook).

Reference computation:
    idx   = codes & 0xFF            (basis row, 256 entries)
    r_q   = (codes >> 8) & 0xFFF
    sign  = bit20 ? -1 : +1
    scale = sign * tanh(r_q / 4095)
    W     = scale[:, None] * basis[idx]        # [8192, 4096]
    y     = x @ W.T                            # [128, 8192]

Key factorization (never materialize the 128MB W):
    Z = x @ basis.T                            # [128, 256]  tiny matmul
    y[b, j] = scale[j] * Z[b, idx[j]]          # column gather + scale

The gather+scale is itself a matmul with a scaled one-hot matrix:
    G[k, j] = scale[j] * (idx[j] == k)         # [256, 1024] per core
    y_core  = Z @ G                            # [128, 1024]
Each one-hot column has a single nonzero, so the matmul computes
scale[j] * Z[b, idx[j]] directly (one product per output).

Sharding: out_features column-parallel across 8 cores (1024 codes per
core); x and basis replicated.  Per core:
    1. stream x^T / basis^T K-tiled as fp16 (halves the memory-roofline
       traffic; ~2^-11 rel err), host pre-laid-out as per-chunk
       contiguous DRAM tensors across three DMA rings; accumulate
       Z [128, 256] in PSUM over 32 fp16 matmuls
    2. decode codes on-chip (DVE bitops + ACT tanh); build G^T tiles
       with one tensor_scalar each ((iota == idx) * scale), PE-transpose
       into G (fp32r) — hidden under the input stream
    3. PE-transpose Z, y = Z^T.T @ G via 4 fp32r matmuls, store fp16
Host reassembles y by concatenating per-core outputs (pure layout).
Overall rel err ~3e-4 (fp16 inputs dominate), vs typical 2e-2 tolerance.
"""

import sys

for _p in ("/opt/trn_rl_repo", "/opt/pypackages"):
    if _p not in sys.path:
        sys.path.insert(0, _p)

import numpy as np

import concourse.bacc as bacc
import concourse.mybir as mybir
import concourse.tile as tile
from concourse.alu_op_type import AluOpType
from concourse.bass_utils import run_bass_kernel_spmd

N_CORES = 8
BATCH = 128
IN_F = 4096
OUT_F = 8192
BASIS = 256
OPC = OUT_F // N_CORES      # 1024 output columns per core
NK = IN_F // 128            # 32 K-tiles
NT = OPC // 128             # 8 code-tiles per core
R_LEVELS = 4095.0

F32 = mybir.dt.float32
F32R = mybir.dt.float32r
BF16 = mybir.dt.bfloat16
FP16 = mybir.dt.float16
I32 = mybir.dt.int32

# K-tiles per input DMA chunk: few big chunks for ring efficiency, small
# final chunk so the PE tail after the last chunk stays small
DMA_CHUNKS = [16, 8, 6, 2]
assert sum(DMA_CHUNKS) == NK

# G^T tiles built after each chunk's matmuls (fills PE DMA-wait gaps)
G_SCHED = {0: [0, 1, 2], 1: [3, 4, 5], 2: [6, 7]}

B_CHUNKS = [(0, 8), (8, 16), (16, 26), (26, 32)]
B_ENGINES = ["sync", "sync", "gpsimd", "gpsimd"]


def build_nc():
    nc = bacc.Bacc(
        "TRN2",
        target_bir_lowering=False,
        debug=False,
        num_devices=N_CORES,
    )

    # fp16 inputs: halves the input traffic (the memory roofline) at
    # ~2^-11 relative error; fp16 range is ample for N(0,1) x and 0.02*N
    # basis, and bf16-class PE rate applies.  One DRAM tensor per DMA
    # chunk so every transfer is fully contiguous in HBM.
    x16_ds = [
        nc.dram_tensor(f"x16c{i}", [128, ch * 128], FP16, kind="ExternalInput")
        for i, ch in enumerate(DMA_CHUNKS)
    ]
    b16_ds = [
        nc.dram_tensor(f"b16c{i}", [128, (be - bs) * 256], FP16,
                       kind="ExternalInput")
        for i, (bs, be) in enumerate(B_CHUNKS)
    ]
    c128_d = nc.dram_tensor("c128", [128, NT], I32, kind="ExternalInput")
    iota_d = nc.dram_tensor("iota", [128, BASIS], F32, kind="ExternalInput")
    ident_d = nc.dram_tensor("ident", [128, 128], F32, kind="ExternalInput")
    out_d = nc.dram_tensor("out", [128, OPC], FP16, kind="ExternalOutput")

    with tile.TileContext(nc) as tc:
        with (
            tc.tile_pool(name="pool", bufs=1) as pool,
            tc.tile_pool(name="zps", bufs=1, space="PSUM") as zps,
            tc.tile_pool(name="tps", bufs=2, space="PSUM") as tps,
            tc.tile_pool(name="yps", bufs=1, space="PSUM") as yps,
        ):
            # ---- small inputs (decode + constants) on the SWDGE ring so
            # the two HWDGE rings start streaming x/basis immediately
            c128 = pool.tile([128, NT], I32)
            nc.gpsimd.dma_start(out=c128[:], in_=c128_d[:])
            iota_bc = pool.tile([128, BASIS], F32)
            nc.gpsimd.dma_start(out=iota_bc[:], in_=iota_d[:])
            ident = pool.tile([128, 128], F32)
            nc.gpsimd.dma_start(out=ident[:], in_=ident_d[:])

            # ---- decode codes -> idx (f32), scale (f32), both [128, NT]
            # (bitVec TSP ops cannot cast dtypes: mask in i32, then cast
            # via fp-ALU mult).  Emitted inside the stream loop (after
            # chunk 0) so the ACT table load for tanh does not delay the
            # scalar ring's first DMA issue.
            idx_f = pool.tile([128, NT], F32)
            scl = pool.tile([128, NT], F32)

            def emit_decode():
                idx_i = pool.tile([128, NT], I32, name="idx_i")
                nc.vector.tensor_scalar(
                    out=idx_i[:], in0=c128[:],
                    scalar1=255, scalar2=None, op0=AluOpType.bitwise_and,
                )
                nc.vector.tensor_scalar_mul(
                    out=idx_f[:], in0=idx_i[:], scalar1=1.0
                )
                rq_i = pool.tile([128, NT], I32, name="rq_i")
                nc.vector.tensor_scalar(
                    out=rq_i[:], in0=c128[:],
                    scalar1=8, scalar2=4095,
                    op0=AluOpType.logical_shift_right,
                    op1=AluOpType.bitwise_and,
                )
                rq = pool.tile([128, NT], F32, name="rq")
                nc.vector.tensor_scalar_mul(
                    out=rq[:], in0=rq_i[:], scalar1=1.0 / R_LEVELS
                )
                th = pool.tile([128, NT], F32, name="th")
                nc.scalar.activation(
                    out=th[:], in_=rq[:],
                    func=mybir.ActivationFunctionType.Tanh,
                )
                sg_i = pool.tile([128, NT], I32, name="sg_i")
                nc.vector.tensor_scalar(
                    out=sg_i[:], in0=c128[:],
                    scalar1=20, scalar2=1,
                    op0=AluOpType.logical_shift_right,
                    op1=AluOpType.bitwise_and,
                )
                sgn = pool.tile([128, NT], F32, name="sgn")
                nc.vector.tensor_scalar(
                    out=sgn[:], in0=sg_i[:],
                    scalar1=-2.0, scalar2=1.0,
                    op0=AluOpType.mult, op1=AluOpType.add,
                )
                nc.vector.tensor_tensor(
                    out=scl[:], in0=th[:], in1=sgn[:], op=AluOpType.mult,
                )

            # ---- G^T tiles: gt[t][p, k] = scale[t*128+p] * (idx[t*128+p]==k)
            # one dual-op tensor_scalar per tile, then PE-transpose into G
            # G_sb[h][k', t*128+j'] with k = h*128+k'.  Emitted interleaved
            # with the stream chunks so the transposes fill PE DMA-wait gaps.
            g_sb = [pool.tile([128, OPC], F32R, tag=f"g{h}", name=f"g_sb{h}") for h in range(2)]

            def emit_g_tile(t):
                gt = pool.tile([128, BASIS], F32, tag="gt", name=f"gt{t}")
                nc.vector.tensor_scalar(
                    out=gt[:], in0=iota_bc[:],
                    scalar1=idx_f[:, t:t + 1], scalar2=scl[:, t:t + 1],
                    op0=AluOpType.is_equal, op1=AluOpType.mult,
                )
                for h in range(2):
                    tp = tps.tile([128, 128], F32, tag="tp", name=f"tp{t}_{h}")
                    nc.tensor.transpose(
                        out=tp[:], in_=gt[:, h * 128:(h + 1) * 128],
                        identity=ident[:],
                    )
                    nc.vector.tensor_copy(
                        out=g_sb[h][:, t * 128:(t + 1) * 128], in_=tp[:]
                    )

            # ---- stream x^T / basis^T (fp16) across THREE DMA rings
            # (sync + gpsimd for basis halves, scalar for x), accumulate
            # Z [128b, 256o] in PSUM (exact fp16 products into fp32 accum)
            x16_sb = pool.tile([128, IN_F], FP16)
            b16_sb = pool.tile([128, 2 * IN_F], FP16)
            z_ps = zps.tile([128, BASIS], F32, tag="z")

            for bi, (bg, bge) in enumerate(B_CHUNKS):
                eng = nc.sync if B_ENGINES[bi] == "sync" else nc.gpsimd
                eng.dma_start(
                    out=b16_sb[:, bg * 256:bge * 256],
                    in_=b16_ds[bi][:],
                )
            g = 0
            for ci, ch in enumerate(DMA_CHUNKS):
                ge = g + ch
                nc.scalar.dma_start(
                    out=x16_sb[:, g * 128:ge * 128],
                    in_=x16_ds[ci][:],
                )
                for n in range(g, ge):
                    nc.tensor.matmul(
                        z_ps[:],
                        lhsT=x16_sb[:, n * 128:(n + 1) * 128],
                        rhs=b16_sb[:, n * 256:(n + 1) * 256],
                        start=(n == 0), stop=(n == NK - 1),
                    )
                if ci == 0:
                    emit_decode()
                for t in G_SCHED.get(ci, []):
                    emit_g_tile(t)
                g = ge

            # Z -> SBUF, PE-transpose into Z^T chunks for the y matmul
            z_sb = pool.tile([128, BASIS], F32)
            nc.vector.tensor_copy(out=z_sb[:], in_=z_ps[:])
            zt = [pool.tile([128, 128], F32R, tag=f"zt{h}", name=f"zt{h}") for h in range(2)]
            for h in range(2):
                ztp = tps.tile([128, 128], F32, tag="tp", name=f"ztp{h}")
                nc.tensor.transpose(
                    out=ztp[:], in_=z_sb[:, h * 128:(h + 1) * 128],
                    identity=ident[:],
                )
                if h == 0:
                    nc.vector.tensor_copy(out=zt[h][:], in_=ztp[:])
                else:
                    nc.scalar.copy(out=zt[h][:], in_=ztp[:])

            # ---- y = Z^T.T @ G, two N-chunks of 512 (fp32r: each one-hot
            # column is a single product, so precision loss is negligible),
            # store each as soon as its PSUM copy lands
            for nch in range(2):
                y_ps = yps.tile([128, 512], F32, tag=f"y{nch}", name=f"y_ps{nch}")
                for h in range(2):
                    nc.tensor.matmul(
                        y_ps[:],
                        lhsT=zt[h][:],
                        rhs=g_sb[h][:, nch * 512:(nch + 1) * 512],
                        start=(h == 0), stop=(h == 1),
                    )
                y_sb = pool.tile([128, 512], FP16, tag=f"ysb{nch}", name=f"y_sb{nch}")
                if nch == 0:
                    nc.vector.tensor_copy(out=y_sb[:], in_=y_ps[:])
                else:
                    nc.scalar.copy(out=y_sb[:], in_=y_ps[:])
                nc.sync.dma_start(
                    out=out_d[:, nch * 512:(nch + 1) * 512], in_=y_sb[:]
                )

    nc.compile()
    return nc


_NC = None


def _get_nc():
    global _NC
    if _NC is None:
        _NC = build_nc()
    return _NC


def make_in_maps(x, codes, basis):
    import ml_dtypes

    bf16 = ml_dtypes.bfloat16
    x = np.ascontiguousarray(x, dtype=np.float32)
    basis = np.ascontiguousarray(basis, dtype=np.float32)
    codes = np.ascontiguousarray(codes, dtype=np.int32)

    # xt[p, n*128 + m] = x[m, n*128 + p]
    xt = np.ascontiguousarray(
        x.reshape(BATCH, NK, 128).transpose(2, 1, 0).reshape(128, IN_F)
    )
    # bt[p, n*256 + o] = basis[o, n*128 + p]
    bt = np.ascontiguousarray(
        basis.reshape(BASIS, NK, 128).transpose(2, 1, 0).reshape(128, 2 * IN_F)
    )
    x16 = xt.astype(np.float16)
    b16 = bt.astype(np.float16)
    xcs, g = {}, 0
    for i, ch in enumerate(DMA_CHUNKS):
        xcs[f"x16c{i}"] = np.ascontiguousarray(x16[:, g * 128:(g + ch) * 128])
        g += ch
    bcs = {}
    for i, (bs, be) in enumerate(B_CHUNKS):
        bcs[f"b16c{i}"] = np.ascontiguousarray(b16[:, bs * 256:be * 256])

    iota = np.ascontiguousarray(
        np.tile(np.arange(BASIS, dtype=np.float32), (128, 1))
    )
    ident = np.eye(128, dtype=np.float32)

    in_maps = []
    for c in range(N_CORES):
        sh = codes[c * OPC:(c + 1) * OPC]
        # wrap-128 layout: c128[p, t] = codes[t*128 + p]
        c128 = np.ascontiguousarray(sh.reshape(NT, 128).T)
        in_maps.append(
            {
                **xcs, **bcs,
                "c128": c128, "iota": iota, "ident": ident,
            }
        )
    return in_maps


def assemble_output(results):
    return np.concatenate(
        [results[c]["out"].astype(np.float32) for c in range(N_CORES)], axis=1
    )


def kernel(x, codes, basis):
    nc = _get_nc()
    in_maps = make_in_maps(x, codes, basis)
    res = run_bass_kernel_spmd(nc, in_maps, list(range(N_CORES)))
    return assemble_output(res.results)


if __name__ == "__main__":
    rng = np.random.default_rng(0)
    x = rng.standard_normal((BATCH, IN_F), dtype=np.float32)
    basis = (rng.standard_normal((BASIS, IN_F)) * 0.02).astype(np.float32)
    codes = rng.integers(0, 1 << 22, size=(OUT_F,), dtype=np.int32)
    y = kernel(x, codes, basis)

    idx = codes & 255
    r = ((codes >> 8) & 4095).astype(np.float32) / R_LEVELS
    sign = np.where(((codes >> 20) & 1) == 1, -1.0, 1.0).astype(np.float32)
    scale = sign * np.tanh(r)
    W = scale[:, None] * basis[idx]
    y_ref = x @ W.T
    err = np.linalg.norm(y - y_ref) / np.linalg.norm(y_ref)
    print("rel err:", err)



# revision 7
# speedup vs baseline: 1.3744x; 1.3744x over previous
"""Trainium2 Bass kernel for nn_BitfieldLinear (vq_codebook).

Reference computation:
    idx   = codes & 0xFF            (basis row, 256 entries)
    r_q   = (codes >> 8) & 0xFFF
    sign  = bit20 ? -1 : +1
    scale = sign * tanh(r_q / 4095)
    W     = scale[:, None] * basis[idx]        # [8192, 4096]
    y     = x @ W.T                            # [128, 8192]

Key factorization (never materialize the 128MB W):
    Z = x @ basis.T                            # [128, 256]  tiny matmul
    y[b, j] = scale[j] * Z[b, idx[j]]          # column gather + scale

Sharding: outputs are binned BY BASIS INDEX — core c owns every output j
with idx[j] // 32 == c (a data-dependent column permutation, undone on
the host).  Then core c only needs basis rows [32c, 32c+32): its Z block
is Z[:, 32c:32c+32] = x @ basis[32c:32c+32].T, computed locally from the
full x (1MB fp16) and a 256KB basis shard — ~1.3MB per core vs 3.25MB
for the replicate-everything layout, with zero cross-core traffic.

Per core:
    1. stream x^T (fp16, 2 HWDGE rings) + basis-shard^T; 32 matmuls
       accumulate Z [128b, 32o] in PSUM (ldweights = x tile, fp16 fast
       weight load; rhs = 32-col basis tile)
    2. decode the ~1024 (padded 1280) binned codes on-chip (DVE bitops +
       ACT tanh); G[k, j] = scale[j] * (idx[j] == 32c+k) built as 10
       one-hot tiles [128, 32] -> PE-transposed into G [32, 1280] fp16
    3. PE-transpose Z -> Z^T [32, 128] fp16; y = Z^T.T @ G in 3 matmuls
       (512/512/256 cols); store fp16
Host reassembles: out[:, bin_cols_c] = y_c[:, :n_c] (pure scatter).
Overall rel err ~1e-3 (fp16 rounding), vs 2e-2 tolerance.
"""

import sys

for _p in ("/opt/trn_rl_repo", "/opt/pypackages"):
    if _p not in sys.path:
        sys.path.insert(0, _p)

import numpy as np

import concourse.bacc as bacc
import concourse.mybir as mybir
import concourse.tile as tile
from concourse.alu_op_type import AluOpType
from concourse.bass_utils import run_bass_kernel_spmd

N_CORES = 8
BATCH = 128
IN_F = 4096
OUT_F = 8192
BASIS = 256
ROWS = BASIS // N_CORES     # 32 basis rows per core
OPC = 1280                  # padded outputs per core (~1024 expected)
NT = OPC // 128             # 10 code-tiles per core
NK = IN_F // 128            # 32 K-tiles
R_LEVELS = 4095.0

F32 = mybir.dt.float32
FP16 = mybir.dt.float16
I32 = mybir.dt.int32

# x chunks: 8 k-tiles (256KB) each; sync ring carries chunks 0-1,
# scalar ring chunks 2-3, each behind its basis half
X_CHUNKS = 4
KPC = NK // X_CHUNKS        # 8 k-tiles per chunk

Y_CHUNKS = [(0, 512), (512, 512), (1024, 256)]


def build_nc():
    nc = bacc.Bacc(
        "TRN2",
        target_bir_lowering=False,
        debug=False,
        num_devices=N_CORES,
    )

    x16_ds = [
        nc.dram_tensor(f"xc{i}", [128, KPC * 128], FP16, kind="ExternalInput")
        for i in range(X_CHUNKS)
    ]
    bA_d = nc.dram_tensor("bA", [128, 16 * ROWS], FP16, kind="ExternalInput")
    bB_d = nc.dram_tensor("bB", [128, 16 * ROWS], FP16, kind="ExternalInput")
    c128_d = nc.dram_tensor("c128", [128, NT], I32, kind="ExternalInput")
    # combo: iota16 [128, 32] (values 32c..32c+31) ++ ident16 [128, 128]
    combo_d = nc.dram_tensor("combo", [128, 160], FP16, kind="ExternalInput")
    out_d = nc.dram_tensor("out", [128, OPC], FP16, kind="ExternalOutput")

    with tile.TileContext(nc) as tc:
        with (
            tc.tile_pool(name="pool", bufs=1) as pool,
            tc.tile_pool(name="zps", bufs=1, space="PSUM") as zps,
            tc.tile_pool(name="tps", bufs=3, space="PSUM") as tps,
            tc.tile_pool(name="yps", bufs=1, space="PSUM") as yps,
        ):
            # ---- DMA issue: small tensors on SWDGE; basis halves lead
            # the two HWDGE rings so the first matmul can start early
            c128 = pool.tile([128, NT], I32)
            nc.gpsimd.dma_start(out=c128[:], in_=c128_d[:])
            combo = pool.tile([128, 160], FP16)
            nc.gpsimd.dma_start(out=combo[:], in_=combo_d[:])

            b_sb = pool.tile([128, NK * ROWS], FP16)
            nc.sync.dma_start(out=b_sb[:, : 16 * ROWS], in_=bA_d[:])
            nc.scalar.dma_start(out=b_sb[:, 16 * ROWS :], in_=bB_d[:])
            x16_sb = pool.tile([128, IN_F], FP16)
            for i in range(X_CHUNKS):
                eng = nc.sync if i < 2 else nc.scalar
                eng.dma_start(
                    out=x16_sb[:, i * KPC * 128 : (i + 1) * KPC * 128],
                    in_=x16_ds[i][:],
                )

            iota16 = combo[:, :ROWS]
            ident16 = combo[:, ROWS : ROWS + 128]

            # ---- decode codes -> idx_f, scl (f32, [128, NT]); is_equal
            # needs f32 scalars.  Emitted after chunk 0's matmuls so the
            # ACT table load for tanh does not delay the scalar ring's
            # DMA issue.
            idx_f = pool.tile([128, NT], F32)
            scl = pool.tile([128, NT], F32)
            iota32 = pool.tile([128, ROWS], F32)

            def emit_decode():
                nc.vector.tensor_scalar_mul(
                    out=iota32[:], in0=iota16, scalar1=1.0
                )
                idx_i = pool.tile([128, NT], I32, name="idx_i")
                nc.vector.tensor_scalar(
                    out=idx_i[:], in0=c128[:],
                    scalar1=255, scalar2=None, op0=AluOpType.bitwise_and,
                )
                nc.vector.tensor_scalar_mul(
                    out=idx_f[:], in0=idx_i[:], scalar1=1.0
                )
                rq_i = pool.tile([128, NT], I32, name="rq_i")
                nc.vector.tensor_scalar(
                    out=rq_i[:], in0=c128[:],
                    scalar1=8, scalar2=4095,
                    op0=AluOpType.logical_shift_right,
                    op1=AluOpType.bitwise_and,
                )
                rq = pool.tile([128, NT], F32, name="rq")
                nc.vector.tensor_scalar_mul(
                    out=rq[:], in0=rq_i[:], scalar1=1.0 / R_LEVELS
                )
                th = pool.tile([128, NT], F32, name="th")
                nc.scalar.activation(
                    out=th[:], in_=rq[:],
                    func=mybir.ActivationFunctionType.Tanh,
                )
                sg_i = pool.tile([128, NT], I32, name="sg_i")
                nc.vector.tensor_scalar(
                    out=sg_i[:], in0=c128[:],
                    scalar1=20, scalar2=1,
                    op0=AluOpType.logical_shift_right,
                    op1=AluOpType.bitwise_and,
                )
                sgn = pool.tile([128, NT], F32, name="sgn")
                nc.vector.tensor_scalar(
                    out=sgn[:], in0=sg_i[:],
                    scalar1=-2.0, scalar2=1.0,
                    op0=AluOpType.mult, op1=AluOpType.add,
                )
                nc.vector.tensor_tensor(
                    out=scl[:], in0=th[:], in1=sgn[:], op=AluOpType.mult,
                )

            # ---- G [32, OPC] fp16: one-hot scaled gather matrix.
            # gt[p, k] = scl[t*128+p] * (idx[t*128+p] == 32c+k), then
            # PE-transpose into G columns t*128..t*128+128.
            g16 = pool.tile([32, OPC], FP16)

            def emit_g_tile(t):
                gt = pool.tile([128, ROWS], FP16, tag=f"gt{t % 3}",
                               name=f"gt{t}")
                nc.vector.tensor_scalar(
                    out=gt[:], in0=iota32[:],
                    scalar1=idx_f[:, t : t + 1], scalar2=scl[:, t : t + 1],
                    op0=AluOpType.is_equal, op1=AluOpType.mult,
                )
                tp = tps.tile([32, 128], FP16, tag="tp", name=f"tp{t}")
                nc.tensor.transpose(out=tp[:], in_=gt[:], identity=ident16)
                if t % 2 == 0:
                    nc.vector.tensor_copy(
                        out=g16[:, t * 128 : (t + 1) * 128], in_=tp[:]
                    )
                else:
                    nc.scalar.copy(
                        out=g16[:, t * 128 : (t + 1) * 128], in_=tp[:]
                    )

            # ---- Z [128b, 32o] = x @ basis_c^T accumulated over 32
            # k-tiles (lhsT = x tile: 128-col fp16 fast weight load;
            # rhs = 32-col basis tile)
            z_ps = zps.tile([128, ROWS], F32, tag="z")
            G_SCHED = {1: [0, 1, 2, 3, 4], 2: [5, 6, 7, 8, 9]}
            for ci in range(X_CHUNKS):
                for n in range(ci * KPC, (ci + 1) * KPC):
                    nc.tensor.matmul(
                        z_ps[:],
                        lhsT=x16_sb[:, n * 128 : (n + 1) * 128],
                        rhs=b_sb[:, n * ROWS : (n + 1) * ROWS],
                        start=(n == 0), stop=(n == NK - 1),
                    )
                if ci == 0:
                    emit_decode()
                for t in G_SCHED.get(ci, []):
                    emit_g_tile(t)

            # ---- Z -> Z^T [32, 128] fp16 via one PE transpose
            z16 = pool.tile([128, ROWS], FP16)
            nc.vector.tensor_copy(out=z16[:], in_=z_ps[:])
            ztp = tps.tile([32, 128], FP16, tag="tp", name="ztp")
            nc.tensor.transpose(out=ztp[:], in_=z16[:], identity=ident16)
            zt16 = pool.tile([32, 128], FP16)
            nc.vector.tensor_copy(out=zt16[:], in_=ztp[:])

            # ---- y = Z^T.T @ G in 3 chunks; store each as its PSUM
            # copy lands
            for q, (off, w) in enumerate(Y_CHUNKS):
                y_ps = yps.tile([128, w], F32, tag=f"y{q}", name=f"y_ps{q}")
                nc.tensor.matmul(
                    y_ps[:],
                    lhsT=zt16[:],
                    rhs=g16[:, off : off + w],
                    start=True, stop=True,
                )
                y_sb = pool.tile([128, w], FP16, tag=f"ysb{q}",
                                 name=f"y_sb{q}")
                if q == 1:
                    nc.scalar.copy(out=y_sb[:], in_=y_ps[:])
                else:
                    nc.vector.tensor_copy(out=y_sb[:], in_=y_ps[:])
                eng = nc.sync if q != 1 else nc.scalar
                eng.dma_start(out=out_d[:, off : off + w], in_=y_sb[:])

    nc.compile()
    return nc


_NC = None


def _get_nc():
    global _NC
    if _NC is None:
        _NC = build_nc()
    return _NC


def make_in_maps(x, codes, basis):
    x = np.ascontiguousarray(x, dtype=np.float32)
    basis = np.ascontiguousarray(basis, dtype=np.float32)
    codes = np.ascontiguousarray(codes, dtype=np.int32)

    # xt[p, n*128 + m] = x[m, n*128 + p]  (shared across cores)
    xt = np.ascontiguousarray(
        x.reshape(BATCH, NK, 128).transpose(2, 1, 0).reshape(128, IN_F)
    ).astype(np.float16)
    xcs = {
        f"xc{i}": np.ascontiguousarray(
            xt[:, i * KPC * 128 : (i + 1) * KPC * 128]
        )
        for i in range(X_CHUNKS)
    }

    idx_all = codes & 255
    bins = idx_all // ROWS
    ident = np.eye(128, dtype=np.float16)

    in_maps = []
    sels = []
    for c in range(N_CORES):
        sel = np.where(bins == c)[0]
        assert len(sel) <= OPC, f"core {c} bin overflow: {len(sel)}"
        sels.append(sel)
        padded = np.zeros(OPC, dtype=np.int32)
        padded[: len(sel)] = codes[sel]
        c128 = np.ascontiguousarray(padded.reshape(NT, 128).T)

        # bt[p, n*32 + o] = basis[32c + o, n*128 + p]
        bt = np.ascontiguousarray(
            basis[c * ROWS : (c + 1) * ROWS]
            .reshape(ROWS, NK, 128)
            .transpose(2, 1, 0)
            .reshape(128, NK * ROWS)
        ).astype(np.float16)

        iota = np.tile(
            (c * ROWS + np.arange(ROWS)).astype(np.float16), (128, 1)
        )
        combo = np.ascontiguousarray(np.concatenate([iota, ident], axis=1))

        in_maps.append(
            {
                **xcs,
                "bA": np.ascontiguousarray(bt[:, : 16 * ROWS]),
                "bB": np.ascontiguousarray(bt[:, 16 * ROWS :]),
                "c128": c128,
                "combo": combo,
            }
        )
    return in_maps, sels


def assemble_output(results, sels):
    out = np.zeros((BATCH, OUT_F), dtype=np.float32)
    for c in range(N_CORES):
        sel = sels[c]
        out[:, sel] = results[c]["out"][:, : len(sel)].astype(np.float32)
    return out


def kernel(x, codes, basis):
    nc = _get_nc()
    in_maps, sels = make_in_maps(x, codes, basis)
    res = run_bass_kernel_spmd(nc, in_maps, list(range(N_CORES)))
    return assemble_output(res.results, sels)


if __name__ == "__main__":
    rng = np.random.default_rng(0)
    x = rng.standard_normal((BATCH, IN_F), dtype=np.float32)
    basis = (rng.standard_normal((BASIS, IN_F)) * 0.02).astype(np.float32)
    codes = rng.integers(0, 1 << 22, size=(OUT_F,), dtype=np.int32)
    y = kernel(x, codes, basis)

    idx = codes & 255
    r = ((codes >> 8) & 4095).astype(np.float32) / R_LEVELS
    sign = np.where(((codes >> 20) & 1) == 1, -1.0, 1.0).astype(np.float32)
    scale = sign * np.tanh(r)
    W = scale[:, None] * basis[idx]
    y_ref = x @ W.T
    err = np.linalg.norm(y - y_ref) / np.linalg.norm(y_ref)
    print("rel err:", err)


# revision 14
# speedup vs baseline: 1.4016x; 1.0198x over previous
"""Trainium2 Bass kernel for nn_BitfieldLinear (vq_codebook).

Reference computation:
    idx   = codes & 0xFF            (basis row, 256 entries)
    r_q   = (codes >> 8) & 0xFFF
    sign  = bit20 ? -1 : +1
    scale = sign * tanh(r_q / 4095)
    W     = scale[:, None] * basis[idx]        # [8192, 4096]
    y     = x @ W.T                            # [128, 8192]

Key factorization (never materialize the 128MB W):
    Z = x @ basis.T                            # [128, 256]  tiny matmul
    y[b, j] = scale[j] * Z[b, idx[j]]          # column gather + scale

Sharding: outputs are binned BY BASIS INDEX — core c owns every output j
with idx[j] // 32 == c (a data-dependent column permutation, undone on
the host).  Core c then only needs basis rows [32c, 32c+32): its Z block
Z[:, 32c:32c+32] = x @ basis[32c:32c+32].T is computed locally from the
full x (1MB fp16) + a 256KB basis shard — ~1.3MB/core vs 3.25MB for the
replicate-everything layout, no cross-core traffic.

Per core:
    1. stream everything fp16 over the two HWDGE rings (5 chunks, basis
       + identity leading); 32 matmuls accumulate Z [128b, 32o] in PSUM
    2. decode the ~1024 (padded 1280) binned codes on-chip: DVE bitops
       + odd-polynomial tanh (no ACT table); scale = tanh(sign*r) since
       tanh is odd.  G[k, j] = scale[j] * (idx[j]-32c == k) built as 10
       one-hot tiles [128, 32] (split gpsimd/vector) -> PE-transposed
       into G [32, 1280] fp16 during the stream's PE gaps
    3. PE-transpose Z -> Z^T [32, 128] fp16; y = Z^T.T @ G in 3 matmuls
       (512/512/256 cols); store fp16
Host reassembles: out[:, bin_cols_c] = y_c[:, :n_c] (pure scatter).
Overall rel err ~1e-3 (fp16 rounding), vs 2e-2 tolerance.
"""

import sys

for _p in ("/opt/trn_rl_repo", "/opt/pypackages"):
    if _p not in sys.path:
        sys.path.insert(0, _p)

import numpy as np

import concourse.bacc as bacc
import concourse.mybir as mybir
import concourse.tile as tile
from concourse.alu_op_type import AluOpType
from concourse.bass_utils import run_bass_kernel_spmd

N_CORES = 8
BATCH = 128
IN_F = 4096
OUT_F = 8192
BASIS = 256
ROWS = BASIS // N_CORES     # 32 basis rows per core
OPC = 1280                  # padded outputs per core (~1024 expected)
NT = OPC // 128             # 10 code-tiles per core
NK = IN_F // 128            # 32 K-tiles
R_LEVELS = 4095.0

# minimax-ish odd polynomial: tanh(s) ~ s*(C0 + C1 s^2 + C2 s^4 + C3 s^6)
# on [-1, 1]; max abs err 8.3e-5
C0, C1, C2, C3 = 0.99974968, -0.32945854, 0.11677167, -0.02555204

F32 = mybir.dt.float32
FP16 = mybir.dt.float16
I32 = mybir.dt.int32

# x k-tile ranges per DMA chunk: m0 (sync) also carries ident+basis;
# sync gets m0+m1, scalar m2+m3+m4.  Decreasing tails so the last
# chunks' completion semaphores fire tight.
X_RANGES = [(0, 4), (4, 11), (11, 19), (19, 26), (26, 32)]
CHUNK_ENG = ["sync", "sync", "scalar", "scalar", "scalar"]

Y_CHUNKS = [(0, 512), (512, 512), (1024, 256)]


def build_nc():
    nc = bacc.Bacc(
        "TRN2",
        target_bir_lowering=False,
        debug=False,
        num_devices=N_CORES,
    )

    c128_d = nc.dram_tensor("c128", [128, NT + 1], I32, kind="ExternalInput")
    m_ds = []
    for i, (ks, ke) in enumerate(X_RANGES):
        w = (ke - ks) * 128 + (128 + NK * ROWS if i == 0 else 0)
        m_ds.append(
            nc.dram_tensor(f"m{i}", [128, w], FP16, kind="ExternalInput")
        )
    out_d = nc.dram_tensor("out", [128, OPC], FP16, kind="ExternalOutput")

    with tile.TileContext(nc) as tc:
        with (
            tc.tile_pool(name="pool", bufs=1) as pool,
            tc.tile_pool(name="zps", bufs=1, space="PSUM") as zps,
            tc.tile_pool(name="tps", bufs=3, space="PSUM") as tps,
            tc.tile_pool(name="yps", bufs=1, space="PSUM") as yps,
        ):
            # ---- DMA issue: tiny codes tensor first (unblocks decode),
            # then the bulk fp16 chunks on both HWDGE rings
            c128 = pool.tile([128, NT + 1], I32)
            nc.sync.dma_start(out=c128[:], in_=c128_d[:])
            m_sb = []
            for i, (ks, ke) in enumerate(X_RANGES):
                t = pool.tile(list(m_ds[i].shape), FP16, name=f"m_sb{i}")
                m_sb.append(t)
                eng = nc.sync if CHUNK_ENG[i] == "sync" else nc.scalar
                eng.dma_start(out=t[:], in_=m_ds[i][:])

            ident16 = m_sb[0][:, :128]

            def xtile(n):
                for i, (ks, ke) in enumerate(X_RANGES):
                    if ks <= n < ke:
                        off = (n - ks) * 128 + (128 + NK * ROWS if i == 0 else 0)
                        return m_sb[i][:, off : off + 128]
                raise AssertionError(n)

            def btile(n):
                return m_sb[0][:, 128 + n * ROWS : 128 + (n + 1) * ROWS]

            # ---- iota 0..31 (f32) generated on-chip, no DMA dependency
            iota_i = pool.tile([128, ROWS], I32)
            nc.gpsimd.iota(iota_i[:], pattern=[[1, ROWS]], channel_multiplier=0)
            iota32 = pool.tile([128, ROWS], F32)
            nc.gpsimd.tensor_scalar_mul(out=iota32[:], in0=iota_i[:], scalar1=1.0)

            # ---- decode: idx_rel (gpsimd) + scale via odd-poly tanh
            # (vector).  base = 32c rides in c128 col NT.
            idx_f = pool.tile([128, NT], F32)
            scl = pool.tile([128, NT], F32)
            rq_i = pool.tile([128, NT], I32, name="rq_i")
            sg_i = pool.tile([128, NT], I32, name="sg_i")

            def emit_decode_vector():
                # bit extraction + AP-scalar ops: DVE-only
                nc.vector.tensor_scalar(
                    out=rq_i[:], in0=c128[:, :NT],
                    scalar1=8, scalar2=4095,
                    op0=AluOpType.logical_shift_right,
                    op1=AluOpType.bitwise_and,
                )
                nc.vector.tensor_scalar(
                    out=sg_i[:], in0=c128[:, :NT],
                    scalar1=20, scalar2=1,
                    op0=AluOpType.logical_shift_right,
                    op1=AluOpType.bitwise_and,
                )
                idx_i = pool.tile([128, NT], I32, name="idx_i")
                nc.vector.tensor_scalar(
                    out=idx_i[:], in0=c128[:, :NT],
                    scalar1=255, scalar2=None, op0=AluOpType.bitwise_and,
                )
                idx_f0 = pool.tile([128, NT], F32, name="idx_f0")
                nc.vector.tensor_scalar_mul(
                    out=idx_f0[:], in0=idx_i[:], scalar1=1.0
                )
                base_f = pool.tile([128, 1], F32, name="base_f")
                nc.vector.tensor_scalar_mul(
                    out=base_f[:], in0=c128[:, NT : NT + 1], scalar1=1.0
                )
                nc.vector.tensor_scalar(
                    out=idx_f[:], in0=idx_f0[:],
                    scalar1=base_f[:], scalar2=None, op0=AluOpType.subtract,
                )

            def emit_decode_gpsimd():
                # fp polynomial tanh chain (const scalars / tensor-tensor
                # only — Pool cannot do AP-scalar ops)
                pm = pool.tile([128, NT], F32, name="pm")
                nc.gpsimd.tensor_scalar(
                    out=pm[:], in0=sg_i[:],
                    scalar1=-2.0, scalar2=1.0,
                    op0=AluOpType.mult, op1=AluOpType.add,
                )
                sr0 = pool.tile([128, NT], F32, name="sr0")
                nc.gpsimd.tensor_scalar_mul(
                    out=sr0[:], in0=rq_i[:], scalar1=1.0 / R_LEVELS
                )
                sr = pool.tile([128, NT], F32, name="sr")
                nc.gpsimd.tensor_tensor(
                    out=sr[:], in0=sr0[:], in1=pm[:], op=AluOpType.mult
                )
                t2 = pool.tile([128, NT], F32, name="t2")
                nc.gpsimd.tensor_tensor(
                    out=t2[:], in0=sr[:], in1=sr[:], op=AluOpType.mult
                )
                u = pool.tile([128, NT], F32, name="u")
                nc.gpsimd.tensor_scalar(
                    out=u[:], in0=t2[:], scalar1=C3, scalar2=C2,
                    op0=AluOpType.mult, op1=AluOpType.add,
                )
                nc.gpsimd.tensor_tensor(
                    out=u[:], in0=u[:], in1=t2[:], op=AluOpType.mult
                )
                nc.gpsimd.tensor_scalar(
                    out=u[:], in0=u[:], scalar1=1.0, scalar2=C1,
                    op0=AluOpType.mult, op1=AluOpType.add,
                )
                nc.gpsimd.tensor_tensor(
                    out=u[:], in0=u[:], in1=t2[:], op=AluOpType.mult
                )
                nc.gpsimd.tensor_scalar(
                    out=u[:], in0=u[:], scalar1=1.0, scalar2=C0,
                    op0=AluOpType.mult, op1=AluOpType.add,
                )
                nc.gpsimd.tensor_tensor(
                    out=scl[:], in0=u[:], in1=sr[:], op=AluOpType.mult
                )

            # ---- G [32, OPC] fp16: gt[p, k] = scl[t*128+p] *
            # (idx_rel[t*128+p] == k); PE-transpose into G cols
            # t*128..t*128+128 (fills PE gaps during the x stream)
            g16 = pool.tile([32, OPC], FP16)
            gts = [
                pool.tile([128, ROWS], FP16, name=f"gt{t}") for t in range(NT)
            ]

            def emit_gt(t, eng):
                eng.tensor_scalar(
                    out=gts[t][:], in0=iota32[:],
                    scalar1=idx_f[:, t : t + 1], scalar2=scl[:, t : t + 1],
                    op0=AluOpType.is_equal, op1=AluOpType.mult,
                )

            def emit_g_transpose(t):
                tp = tps.tile([32, 128], FP16, tag="tp", name=f"tp{t}")
                nc.tensor.transpose(out=tp[:], in_=gts[t][:], identity=ident16)
                nc.scalar.copy(
                    out=g16[:, t * 128 : (t + 1) * 128], in_=tp[:]
                )

            # ---- Z^T [32o, 128b] accumulated directly over 32 k-tiles
            # (lhsT = 32-col basis tile, rhs = 128-col x tile) — no
            # transpose needed before the y matmul
            z_ps = zps.tile([ROWS, 128], F32, tag="z")
            for ci, (ks, ke) in enumerate(X_RANGES):
                for n in range(ks, ke):
                    nc.tensor.matmul(
                        z_ps[:],
                        lhsT=btile(n),
                        rhs=xtile(n),
                        start=(n == 0), stop=(n == NK - 1),
                    )
                if ci == 0:
                    emit_decode_vector()
                    emit_decode_gpsimd()
                elif ci == 1:
                    for t in range(NT):
                        emit_gt(t, nc.vector)
                elif ci == 2:
                    for t in range(0, 5):
                        emit_g_transpose(t)
                elif ci == 3:
                    for t in range(5, NT):
                        emit_g_transpose(t)

            zt16 = pool.tile([32, 128], FP16)
            nc.vector.tensor_copy(out=zt16[:], in_=z_ps[:])

            # ---- y = Z^T.T @ G in 3 chunks; store each as it lands
            y16 = pool.tile([128, OPC], FP16)
            for q, (off, w) in enumerate(Y_CHUNKS):
                y_ps = yps.tile([128, w], F32, tag=f"y{q}", name=f"y_ps{q}")
                nc.tensor.matmul(
                    y_ps[:],
                    lhsT=zt16[:],
                    rhs=g16[:, off : off + w],
                    start=True, stop=True,
                )
                if q == 1:
                    nc.scalar.copy(out=y16[:, off : off + w], in_=y_ps[:])
                else:
                    nc.vector.tensor_copy(
                        out=y16[:, off : off + w], in_=y_ps[:]
                    )
                eng = nc.sync if q != 1 else nc.scalar
                eng.dma_start(
                    out=out_d[:, off : off + w], in_=y16[:, off : off + w]
                )

    nc.compile()
    return nc


_NC = None


def _get_nc():
    global _NC
    if _NC is None:
        _NC = build_nc()
    return _NC


def make_in_maps(x, codes, basis):
    x = np.ascontiguousarray(x, dtype=np.float32)
    basis = np.ascontiguousarray(basis, dtype=np.float32)
    codes = np.ascontiguousarray(codes, dtype=np.int32)

    # xt[p, n*128 + m] = x[m, n*128 + p]  (shared across cores)
    xt = np.ascontiguousarray(
        x.reshape(BATCH, NK, 128).transpose(2, 1, 0).reshape(128, IN_F)
    ).astype(np.float16)
    xslices = [xt[:, ks * 128 : ke * 128] for ks, ke in X_RANGES]
    ident = np.eye(128, dtype=np.float16)

    idx_all = codes & 255
    bins = idx_all // ROWS

    in_maps = []
    sels = []
    for c in range(N_CORES):
        sel = np.where(bins == c)[0]
        assert len(sel) <= OPC, f"core {c} bin overflow: {len(sel)}"
        sels.append(sel)
        padded = np.zeros(OPC, dtype=np.int32)
        padded[: len(sel)] = codes[sel]
        c128 = np.empty((128, NT + 1), dtype=np.int32)
        c128[:, :NT] = padded.reshape(NT, 128).T
        c128[:, NT] = c * ROWS

        # bt[p, n*32 + o] = basis[32c + o, n*128 + p]
        bt = np.ascontiguousarray(
            basis[c * ROWS : (c + 1) * ROWS]
            .reshape(ROWS, NK, 128)
            .transpose(2, 1, 0)
            .reshape(128, NK * ROWS)
        ).astype(np.float16)

        im = {"c128": c128}
        for i in range(len(X_RANGES)):
            if i == 0:
                im["m0"] = np.ascontiguousarray(
                    np.concatenate([ident, bt, xslices[0]], axis=1)
                )
            else:
                im[f"m{i}"] = np.ascontiguousarray(xslices[i])
        in_maps.append(im)
    return in_maps, sels


def assemble_output(results, sels):
    out = np.zeros((BATCH, OUT_F), dtype=np.float32)
    for c in range(N_CORES):
        sel = sels[c]
        out[:, sel] = results[c]["out"][:, : len(sel)].astype(np.float32)
    return out


def kernel(x, codes, basis):
    nc = _get_nc()
    in_maps, sels = make_in_maps(x, codes, basis)
    res = run_bass_kernel_spmd(nc, in_maps, list(range(N_CORES)))
    return assemble_output(res.results, sels)


if __name__ == "__main__":
    rng = np.random.default_rng(0)
    x = rng.standard_normal((BATCH, IN_F), dtype=np.float32)
    basis = (rng.standard_normal((BASIS, IN_F)) * 0.02).astype(np.float32)
    codes = rng.integers(0, 1 << 22, size=(OUT_F,), dtype=np.int32)
    y = kernel(x, codes, basis)

    idx = codes & 255
    r = ((codes >> 8) & 4095).astype(np.float32) / R_LEVELS
    sign = np.where(((codes >> 20) & 1) == 1, -1.0, 1.0).astype(np.float32)
    scale = sign * np.tanh(r)
    W = scale[:, None] * basis[idx]
    y_ref = x @ W.T
    err = np.linalg.norm(y - y_ref) / np.linalg.norm(y_ref)
    print("rel err:", err)


# revision 17
# speedup vs baseline: 1.4492x; 1.0340x over previous
"""Trainium2 Bass kernel for nn_BitfieldLinear (vq_codebook).

Reference computation:
    idx   = codes & 0xFF            (basis row, 256 entries)
    r_q   = (codes >> 8) & 0xFFF
    sign  = bit20 ? -1 : +1
    scale = sign * tanh(r_q / 4095)
    W     = scale[:, None] * basis[idx]        # [8192, 4096]
    y     = x @ W.T                            # [128, 8192]

Key factorization (never materialize the 128MB W):
    Z = x @ basis.T                            # [128, 256]  tiny matmul
    y[b, j] = scale[j] * Z[b, idx[j]]          # column gather + scale

Sharding: outputs are binned BY BASIS INDEX — core c owns every output j
with idx[j] // 32 == c (a data-dependent column permutation, undone on
the host).  Core c then only needs basis rows [32c, 32c+32): its Z block
Z[:, 32c:32c+32] = x @ basis[32c:32c+32].T is computed locally from the
full x (1MB fp16) + a 256KB basis shard — ~1.3MB/core vs 3.25MB for the
replicate-everything layout, no cross-core traffic.

Per core:
    1. stream everything fp16 over the two HWDGE rings (5 chunks, basis
       + identity leading); 32 matmuls accumulate Z [128b, 32o] in PSUM
    2. decode the ~1024 (padded 1280) binned codes on-chip: DVE bitops
       + odd-polynomial tanh (no ACT table); scale = tanh(sign*r) since
       tanh is odd.  G[k, j] = scale[j] * (idx[j]-32c == k) built as 10
       one-hot tiles [128, 32] (split gpsimd/vector) -> PE-transposed
       into G [32, 1280] fp16 during the stream's PE gaps
    3. PE-transpose Z -> Z^T [32, 128] fp16; y = Z^T.T @ G in 3 matmuls
       (512/512/256 cols); store fp16
Host reassembles: out[:, bin_cols_c] = y_c[:, :n_c] (pure scatter).
Overall rel err ~1e-3 (fp16 rounding), vs 2e-2 tolerance.
"""

import sys

for _p in ("/opt/trn_rl_repo", "/opt/pypackages"):
    if _p not in sys.path:
        sys.path.insert(0, _p)

import numpy as np

import concourse.bacc as bacc
import concourse.mybir as mybir
import concourse.tile as tile
from concourse.alu_op_type import AluOpType
from concourse.bass_utils import run_bass_kernel_spmd

N_CORES = 8
BATCH = 128
IN_F = 4096
OUT_F = 8192
BASIS = 256
ROWS = BASIS // N_CORES     # 32 basis rows per core
OPC = 1280                  # padded outputs per core (~1024 expected)
NT = OPC // 128             # 10 code-tiles per core
NK = IN_F // 128            # 32 K-tiles
R_LEVELS = 4095.0

# minimax-ish odd polynomial: tanh(s) ~ s*(C0 + C1 s^2 + C2 s^4 + C3 s^6)
# on [-1, 1]; max abs err 8.3e-5
C0, C1, C2, C3 = 0.99974968, -0.32945854, 0.11677167, -0.02555204

F32 = mybir.dt.float32
FP16 = mybir.dt.float16
I32 = mybir.dt.int32

# x k-tile ranges per DMA chunk: m0 (sync) also carries ident+basis;
# sync gets m0+m1, scalar m2+m3+m4.  Decreasing tails so the last
# chunks' completion semaphores fire tight.
X_RANGES = [(0, 4), (4, 11), (11, 19), (19, 26), (26, 32)]
CHUNK_ENG = ["sync", "sync", "scalar", "scalar", "scalar"]

Y_CHUNKS = [(0, 512), (512, 512), (1024, 256)]


def build_nc():
    nc = bacc.Bacc(
        "TRN2",
        target_bir_lowering=False,
        debug=False,
        num_devices=N_CORES,
    )

    c128_d = nc.dram_tensor("c128", [128, NT + 1], I32, kind="ExternalInput")
    m_ds = []
    for i, (ks, ke) in enumerate(X_RANGES):
        w = (ke - ks) * 128 + (128 + NK * ROWS if i == 0 else 0)
        m_ds.append(
            nc.dram_tensor(f"m{i}", [128, w], FP16, kind="ExternalInput")
        )
    out_d = nc.dram_tensor("out", [128, OPC], FP16, kind="ExternalOutput")

    with tile.TileContext(nc) as tc:
        with (
            tc.tile_pool(name="pool", bufs=1) as pool,
            tc.tile_pool(name="zps", bufs=1, space="PSUM") as zps,
            tc.tile_pool(name="tps", bufs=2, space="PSUM") as tps,
            tc.tile_pool(name="yps", bufs=1, space="PSUM") as yps,
        ):
            # ---- DMA issue: tiny codes tensor first (unblocks decode),
            # then the bulk fp16 chunks on both HWDGE rings
            c128 = pool.tile([128, NT + 1], I32)
            nc.sync.dma_start(out=c128[:], in_=c128_d[:])
            m_sb = []
            for i, (ks, ke) in enumerate(X_RANGES):
                t = pool.tile(list(m_ds[i].shape), FP16, name=f"m_sb{i}")
                m_sb.append(t)
                eng = nc.sync if CHUNK_ENG[i] == "sync" else nc.scalar
                eng.dma_start(out=t[:], in_=m_ds[i][:])

            ident16 = m_sb[0][:, :128]

            def xtile(n):
                for i, (ks, ke) in enumerate(X_RANGES):
                    if ks <= n < ke:
                        off = (n - ks) * 128 + (128 + NK * ROWS if i == 0 else 0)
                        return m_sb[i][:, off : off + 128]
                raise AssertionError(n)

            def btile(n):
                return m_sb[0][:, 128 + n * ROWS : 128 + (n + 1) * ROWS]

            # ---- iota 0..31 (f32) generated on-chip, no DMA dependency
            iota_i = pool.tile([128, ROWS], I32)
            nc.gpsimd.iota(iota_i[:], pattern=[[1, ROWS]], channel_multiplier=0)
            iota32 = pool.tile([128, ROWS], F32)
            nc.gpsimd.tensor_scalar_mul(out=iota32[:], in0=iota_i[:], scalar1=1.0)

            # ---- decode: idx_rel (gpsimd) + scale via odd-poly tanh
            # (vector).  base = 32c rides in c128 col NT.
            idx_f = pool.tile([128, NT], F32)
            scl = pool.tile([128, NT], F32)
            rq_i = pool.tile([128, NT], I32, name="rq_i")
            sg_i = pool.tile([128, NT], I32, name="sg_i")

            def emit_decode_vector():
                # bit extraction + AP-scalar ops: DVE-only
                nc.vector.tensor_scalar(
                    out=rq_i[:], in0=c128[:, :NT],
                    scalar1=8, scalar2=4095,
                    op0=AluOpType.logical_shift_right,
                    op1=AluOpType.bitwise_and,
                )
                nc.vector.tensor_scalar(
                    out=sg_i[:], in0=c128[:, :NT],
                    scalar1=20, scalar2=1,
                    op0=AluOpType.logical_shift_right,
                    op1=AluOpType.bitwise_and,
                )
                idx_i = pool.tile([128, NT], I32, name="idx_i")
                nc.vector.tensor_scalar(
                    out=idx_i[:], in0=c128[:, :NT],
                    scalar1=255, scalar2=None, op0=AluOpType.bitwise_and,
                )
                idx_f0 = pool.tile([128, NT], F32, name="idx_f0")
                nc.vector.tensor_scalar_mul(
                    out=idx_f0[:], in0=idx_i[:], scalar1=1.0
                )
                base_f = pool.tile([128, 1], F32, name="base_f")
                nc.vector.tensor_scalar_mul(
                    out=base_f[:], in0=c128[:, NT : NT + 1], scalar1=1.0
                )
                nc.vector.tensor_scalar(
                    out=idx_f[:], in0=idx_f0[:],
                    scalar1=base_f[:], scalar2=None, op0=AluOpType.subtract,
                )

            def emit_decode_poly():
                # fp polynomial tanh chain on DVE
                pm = pool.tile([128, NT], F32, name="pm")
                nc.vector.tensor_scalar(
                    out=pm[:], in0=sg_i[:],
                    scalar1=-2.0, scalar2=1.0,
                    op0=AluOpType.mult, op1=AluOpType.add,
                )
                sr0 = pool.tile([128, NT], F32, name="sr0")
                nc.vector.tensor_scalar_mul(
                    out=sr0[:], in0=rq_i[:], scalar1=1.0 / R_LEVELS
                )
                sr = pool.tile([128, NT], F32, name="sr")
                nc.vector.tensor_tensor(
                    out=sr[:], in0=sr0[:], in1=pm[:], op=AluOpType.mult
                )
                t2 = pool.tile([128, NT], F32, name="t2")
                nc.vector.tensor_tensor(
                    out=t2[:], in0=sr[:], in1=sr[:], op=AluOpType.mult
                )
                u = pool.tile([128, NT], F32, name="u")
                nc.vector.tensor_scalar(
                    out=u[:], in0=t2[:], scalar1=C3, scalar2=C2,
                    op0=AluOpType.mult, op1=AluOpType.add,
                )
                nc.vector.tensor_tensor(
                    out=u[:], in0=u[:], in1=t2[:], op=AluOpType.mult
                )
                nc.vector.tensor_scalar(
                    out=u[:], in0=u[:], scalar1=1.0, scalar2=C1,
                    op0=AluOpType.mult, op1=AluOpType.add,
                )
                nc.vector.tensor_tensor(
                    out=u[:], in0=u[:], in1=t2[:], op=AluOpType.mult
                )
                nc.vector.tensor_scalar(
                    out=u[:], in0=u[:], scalar1=1.0, scalar2=C0,
                    op0=AluOpType.mult, op1=AluOpType.add,
                )
                nc.vector.tensor_tensor(
                    out=scl[:], in0=u[:], in1=sr[:], op=AluOpType.mult
                )

            # ---- G [32, OPC] fp16: gt[p, k] = scl[t*128+p] *
            # (idx_rel[t*128+p] == k); PE-transpose into G cols
            # t*128..t*128+128 (fills PE gaps during the x stream)
            g16 = pool.tile([32, OPC], FP16)
            gts = [
                pool.tile([128, ROWS], FP16, name=f"gt{t}") for t in range(NT)
            ]

            def emit_gt(t, eng):
                eng.tensor_scalar(
                    out=gts[t][:], in0=iota32[:],
                    scalar1=idx_f[:, t : t + 1], scalar2=scl[:, t : t + 1],
                    op0=AluOpType.is_equal, op1=AluOpType.mult,
                )

            def emit_g_transpose(t):
                tp = tps.tile([32, 128], FP16, tag="tp", name=f"tp{t}")
                nc.tensor.transpose(out=tp[:], in_=gts[t][:], identity=ident16)
                nc.scalar.copy(
                    out=g16[:, t * 128 : (t + 1) * 128], in_=tp[:]
                )

            # ---- PE warm-up: ~40 dummy matmuls on a memset scratch
            # during the pre-stream idle window so the HAM clock gate
            # ramps the PE to 2.4 GHz before the real matmuls arrive
            warm_sb = pool.tile([128, 128], FP16, name="warm_sb")
            nc.gpsimd.memset(warm_sb[:], 0.0)
            warm_ps = zps.tile([128, 128], F32, tag="warm", name="warm_ps")
            for wi in range(40):
                nc.tensor.matmul(
                    warm_ps[:], lhsT=warm_sb[:], rhs=warm_sb[:],
                    start=True, stop=True,
                )

            # ---- Z^T [32o, 128b] accumulated directly over the 32
            # k-tiles (lhsT = 32-col basis tile, rhs = 128-col x tile —
            # no transpose needed before the y matmul), split into an A
            # phase (chunks m0-m3) and a B phase (m4) so only the last 6
            # k-tiles serialize behind the final input DMA
            B_START = X_RANGES[-1][0]
            zA_ps = zps.tile([ROWS, 128], F32, tag="zA")
            zB_ps = zps.tile([ROWS, 128], F32, tag="zB")
            for ci, (ks, ke) in enumerate(X_RANGES[:-1]):
                for n in range(ks, ke):
                    nc.tensor.matmul(
                        zA_ps[:],
                        lhsT=btile(n),
                        rhs=xtile(n),
                        start=(n == 0), stop=(n == B_START - 1),
                    )
                if ci == 0:
                    emit_decode_vector()
                    emit_decode_poly()
                elif ci == 1:
                    for t in range(NT):
                        emit_gt(t, nc.vector)
                    for t in range(NT):
                        emit_g_transpose(t)

            ztA16 = pool.tile([32, 128], FP16, name="ztA16")
            nc.vector.tensor_copy(out=ztA16[:], in_=zA_ps[:])

            # ---- y = Z^T.T @ G in 3 column chunks, accumulating the A
            # then B phase contributions in PSUM
            y16 = pool.tile([128, OPC], FP16)
            y_pss = []
            for q, (off, w) in enumerate(Y_CHUNKS):
                y_ps = yps.tile([128, w], F32, tag=f"y{q}", name=f"y_ps{q}")
                y_pss.append(y_ps)
                nc.tensor.matmul(
                    y_ps[:],
                    lhsT=ztA16[:],
                    rhs=g16[:, off : off + w],
                    start=True, stop=False,
                )

            for n in range(B_START, NK):
                nc.tensor.matmul(
                    zB_ps[:],
                    lhsT=btile(n),
                    rhs=xtile(n),
                    start=(n == B_START), stop=(n == NK - 1),
                )
            ztB16 = pool.tile([32, 128], FP16, name="ztB16")
            nc.vector.tensor_copy(out=ztB16[:], in_=zB_ps[:])

            for q, (off, w) in enumerate(Y_CHUNKS):
                nc.tensor.matmul(
                    y_pss[q][:],
                    lhsT=ztB16[:],
                    rhs=g16[:, off : off + w],
                    start=False, stop=True,
                )
                if q == 1:
                    nc.scalar.copy(out=y16[:, off : off + w], in_=y_pss[q][:])
                else:
                    nc.vector.tensor_copy(
                        out=y16[:, off : off + w], in_=y_pss[q][:]
                    )
            # single output store: one issue, one completion receipt,
            # and keeps total DMA count within the 8 semaphore lanes
            nc.sync.dma_start(out=out_d[:], in_=y16[:])

    nc.compile()
    return nc


_NC = None


def _get_nc():
    global _NC
    if _NC is None:
        _NC = build_nc()
    return _NC


def make_in_maps(x, codes, basis):
    x = np.ascontiguousarray(x, dtype=np.float32)
    basis = np.ascontiguousarray(basis, dtype=np.float32)
    codes = np.ascontiguousarray(codes, dtype=np.int32)

    # xt[p, n*128 + m] = x[m, n*128 + p]  (shared across cores)
    xt = np.ascontiguousarray(
        x.reshape(BATCH, NK, 128).transpose(2, 1, 0).reshape(128, IN_F)
    ).astype(np.float16)
    xslices = [xt[:, ks * 128 : ke * 128] for ks, ke in X_RANGES]
    ident = np.eye(128, dtype=np.float16)

    idx_all = codes & 255
    bins = idx_all // ROWS

    in_maps = []
    sels = []
    for c in range(N_CORES):
        sel = np.where(bins == c)[0]
        assert len(sel) <= OPC, f"core {c} bin overflow: {len(sel)}"
        sels.append(sel)
        padded = np.zeros(OPC, dtype=np.int32)
        padded[: len(sel)] = codes[sel]
        c128 = np.empty((128, NT + 1), dtype=np.int32)
        c128[:, :NT] = padded.reshape(NT, 128).T
        c128[:, NT] = c * ROWS

        # bt[p, n*32 + o] = basis[32c + o, n*128 + p]
        bt = np.ascontiguousarray(
            basis[c * ROWS : (c + 1) * ROWS]
            .reshape(ROWS, NK, 128)
            .transpose(2, 1, 0)
            .reshape(128, NK * ROWS)
        ).astype(np.float16)

        im = {"c128": c128}
        for i in range(len(X_RANGES)):
            if i == 0:
                im["m0"] = np.ascontiguousarray(
                    np.concatenate([ident, bt, xslices[0]], axis=1)
                )
            else:
                im[f"m{i}"] = np.ascontiguousarray(xslices[i])
        in_maps.append(im)
    return in_maps, sels


def assemble_output(results, sels):
    out = np.zeros((BATCH, OUT_F), dtype=np.float32)
    for c in range(N_CORES):
        sel = sels[c]
        out[:, sel] = results[c]["out"][:, : len(sel)].astype(np.float32)
    return out


def kernel(x, codes, basis):
    nc = _get_nc()
    in_maps, sels = make_in_maps(x, codes, basis)
    res = run_bass_kernel_spmd(nc, in_maps, list(range(N_CORES)))
    return assemble_output(res.results, sels)


if __name__ == "__main__":
    rng = np.random.default_rng(0)
    x = rng.standard_normal((BATCH, IN_F), dtype=np.float32)
    basis = (rng.standard_normal((BASIS, IN_F)) * 0.02).astype(np.float32)
    codes = rng.integers(0, 1 << 22, size=(OUT_F,), dtype=np.int32)
    y = kernel(x, codes, basis)

    idx = codes & 255
    r = ((codes >> 8) & 4095).astype(np.float32) / R_LEVELS
    sign = np.where(((codes >> 20) & 1) == 1, -1.0, 1.0).astype(np.float32)
    scale = sign * np.tanh(r)
    W = scale[:, None] * basis[idx]
    y_ref = x @ W.T
    err = np.linalg.norm(y - y_ref) / np.linalg.norm(y_ref)
    print("rel err:", err)
